# revision 1
# baseline (speedup 1.0000x reference)
"""ChildSum TreeLSTM (complete binary trees, heap layout) on 8 Trainium2 cores.

Strategy
--------
Data-parallel over the tree batch: 256 trees -> 32 per core.  All on-chip
tensors live in a feature-major ("transposed") layout: partitions = one
128-feature chunk (2 chunks cover D=256), free dim = (node, batch) columns.
The host pre-packs x into this layout (fp16), so the device never transposes
anything and every matmul contracts over the partition dim directly:

    iou^T[m-chunk, cols] = sum_k W_iou^T[k, m].T @ x^T[k, cols]
                         + sum_k U_iou^T[k, m].T @ h_sum^T[k, cols]

Levels run bottom-up.  The big levels (9..6) are processed per 4-tree chunk
(8 chunks, pipelined); the small levels (5..0) are processed once for all 32
trees ("merged" phase) so the serial top-of-tree tail is paid once per core
instead of once per chunk.  Each level is processed in <=512-column blocks:
PE fills 2-bank PSUM tiles (i, o, u, f_even, f_odd), ACT applies
sigmoid/tanh out of PSUM into fp16 SBUF, DVE runs the c/h elementwise chain
with even/odd strided views of the child level.  fp16 on-chip math with
fp32 PSUM accumulation; fp32 root outputs.
"""

from contextlib import ExitStack

import numpy as np

# Problem constants (hardcoded; kernel.py must be self-contained).
B = 256
DEPTH = 9
NNODES = 2 ** (DEPTH + 1) - 1  # 1023
D = 256
NCORES = 8
B_LOC = B // NCORES            # 32 trees per core
B_C = 4                        # trees per chunk (levels 9..6)
NCH = B_LOC // B_C             # chunks per core
COLS = NNODES * B_C            # x columns per chunk
MERGE_LVL = 6                  # levels < MERGE_LVL run merged over all 32 trees
NM = 2 ** MERGE_LVL - 1        # 63 nodes in merged levels (heap prefix)

TRACE = False
LAST = {}

_NC_CACHE = {}


def _build(nch, has_bias):
    """Emit the Bass program for one core processing `nch` chunks."""
    import concourse.mybir as mybir
    import concourse.tile as tile
    from concourse import bacc

    f16 = mybir.dt.float16
    f32 = mybir.dt.float32
    Sig = mybir.ActivationFunctionType.Sigmoid
    Tanh = mybir.ActivationFunctionType.Tanh

    nb = nch * B_C  # merged batch (32 for the full kernel)

    nc = bacc.Bacc(enable_partition_id=False)

    xt = nc.declare_dram_parameter("xt", [nch, 2, 128, COLS], f16, isOutput=False)
    xm_d = nc.declare_dram_parameter("xm", [2, 128, NM * nb], f16, isOutput=False)
    wiou_d = nc.declare_dram_parameter("wiou", [2, 128, 768], f16, isOutput=False)
    uiou_d = nc.declare_dram_parameter("uiou", [2, 128, 768], f16, isOutput=False)
    wf_d = nc.declare_dram_parameter("wf", [2, 128, 256], f16, isOutput=False)
    uf_d = nc.declare_dram_parameter("uf", [2, 128, 256], f16, isOutput=False)
    if has_bias:
        biou_d = nc.declare_dram_parameter("biou", [768], f32, isOutput=False)
        bf_d = nc.declare_dram_parameter("bf", [256], f32, isOutput=False)
    hout = nc.declare_dram_parameter("hout", [2, 128, nb], f32, isOutput=True)
    cout = nc.declare_dram_parameter("cout", [2, 128, nb], f32, isOutput=True)

    with tile.TileContext(nc) as tc, ExitStack() as ctx:
        wpool = ctx.enter_context(tc.tile_pool(name="wpool", bufs=1))
        big = ctx.enter_context(tc.tile_pool(name="big", bufs=2))
        trans = ctx.enter_context(tc.tile_pool(name="trans", bufs=2))
        xpool = ctx.enter_context(tc.tile_pool(name="xpool", bufs=2))
        pp = ctx.enter_context(tc.tile_pool(name="pp", bufs=4, space="PSUM"))

        # Dummy activation up front so the sigmoid/tanh table-set load
        # (~2.7us) overlaps the initial weight DMAs instead of stalling the
        # first leaf block (sigmoid_and_others contains tanh too).
        warm = wpool.tile([128, 1], f32, tag="warm")
        nc.vector.memset(warm, 0.0)
        nc.scalar.activation(out=warm, in_=warm, func=Sig)

        wiou_sb = wpool.tile([128, 2, 768], f16, tag="wiou")
        uiou_sb = wpool.tile([128, 2, 768], f16, tag="uiou")
        wf_sb = wpool.tile([128, 2, 256], f16, tag="wf")
        uf_sb = wpool.tile([128, 2, 256], f16, tag="uf")
        for kc in range(2):
            nc.sync.dma_start(out=wiou_sb[:, kc, :], in_=wiou_d[kc])
            nc.sync.dma_start(out=uiou_sb[:, kc, :], in_=uiou_d[kc])
            nc.sync.dma_start(out=wf_sb[:, kc, :], in_=wf_d[kc])
            nc.sync.dma_start(out=uf_sb[:, kc, :], in_=uf_d[kc])
        biou_sb = bf_sb = None
        if has_bias:
            biou_sb = wpool.tile([128, 6], f32, tag="biou")
            bf_sb = wpool.tile([128, 2], f32, tag="bf")
            for mj in range(6):
                nc.sync.dma_start(
                    out=biou_sb[:, mj : mj + 1],
                    in_=biou_d[mj * 128 : (mj + 1) * 128].rearrange(
                        "(p one) -> p one", one=1
                    ),
                )
            for mj in range(2):
                nc.sync.dma_start(
                    out=bf_sb[:, mj : mj + 1],
                    in_=bf_d[mj * 128 : (mj + 1) * 128].rearrange(
                        "(p one) -> p one", one=1
                    ),
                )

        def act(out_t, in_t, func, bias_sb, bias_cols):
            if not has_bias or bias_sb is None:
                nc.scalar.activation(out=out_t, in_=in_t, func=func)
            else:
                for kk in range(2):
                    col = bias_cols[kk]
                    nc.scalar.activation(
                        out=out_t[:, kk, :],
                        in_=in_t[:, kk, :],
                        func=func,
                        bias=bias_sb[:, col : col + 1],
                    )

        def emit_block(P, bo, bc, xsl, hs_in, h_child, c_child, h_l, c_l, hs_out):
            """One <=512-column block at column offset `bo` of its level.

            P: columns in block; bc: batch stride (trees per node group);
            xsl(kc) -> [128, P] x slice; hs_in: child-sum tile or None (leaf);
            h_child/c_child: child-level tiles [128, 2, 2R] or None (leaf);
            h_l/c_l: this level's output tiles; hs_out: next child-sum tile
            (written here) or None.
            """
            leaf = h_child is None
            # For small blocks a [128,4,P] fp32 tile still fits the 2-bank
            # PSUM slot, so i+o (and f_even+f_odd) share one tile and one
            # fused ACT instruction.
            fuse4 = P <= 256

            def mm_fill(pt, mj, mcol_base, w_sb, extra):
                terms = [
                    (w_sb[:, kc, (mcol_base + (mj % 2)) * 128 :
                          (mcol_base + (mj % 2) + 1) * 128], xsl(kc))
                    for kc in range(2)
                ]
                terms += extra(mj % 2)
                for t_i, (lw, lrhs) in enumerate(terms):
                    nc.tensor.matmul(
                        pt[:, mj, :], lw, lrhs,
                        start=(t_i == 0), stop=(t_i == len(terms) - 1),
                    )

            def mm_group(mcol_base, w_sb, extra):
                pt = pp.tile([128, 2, P], f32, tag="ps", name="pt")
                for mj in range(2):
                    mm_fill(pt, mj, mcol_base, w_sb, extra)
                return pt

            def child_view(t, kc, par):
                v = t[:, kc, 2 * bo : 2 * bo + 2 * P].rearrange(
                    "p (q two b) -> p q two b", two=2, b=bc
                )
                return v[:, :, par, :]

            if leaf:
                no_extra = lambda mj: []
            else:
                def iou_extra(base):
                    def ex(mj):
                        return [
                            (uiou_sb[:, kc, (base + mj) * 128 :
                                     (base + mj + 1) * 128],
                             hs_in[:, kc, bo : bo + P])
                            for kc in range(2)
                        ]
                    return ex

                def f_extra(par):
                    def ex(mj):
                        return [
                            (uf_sb[:, kc, mj * 128 : (mj + 1) * 128],
                             child_view(h_child, kc, par))
                            for kc in range(2)
                        ]
                    return ex

            i_extra = no_extra if leaf else iou_extra(0)
            o_extra = no_extra if leaf else iou_extra(2)
            u_extra = no_extra if leaf else iou_extra(4)

            if fuse4:
                io_ps = pp.tile([128, 4, P], f32, tag="ps", name="io_ps")
                for mj in range(2):
                    mm_fill(io_ps, mj, 0, wiou_sb, i_extra)
                for mj in range(2, 4):
                    mm_fill(io_ps, mj, 2, wiou_sb, o_extra)
                u_ps = mm_group(4, wiou_sb, u_extra)
                io_sb = trans.tile([128, 4, P], f16, tag="isb", name="io_sb")
                if has_bias:
                    for kk in range(2):
                        nc.scalar.activation(
                            out=io_sb[:, kk, :], in_=io_ps[:, kk, :], func=Sig,
                            bias=biou_sb[:, kk : kk + 1])
                        nc.scalar.activation(
                            out=io_sb[:, 2 + kk, :], in_=io_ps[:, 2 + kk, :],
                            func=Sig, bias=biou_sb[:, 2 + kk : 3 + kk])
                else:
                    nc.scalar.activation(out=io_sb, in_=io_ps, func=Sig)
                i_sb = io_sb[:, 0:2, :]
                o_sb = io_sb[:, 2:4, :]
            else:
                i_ps = mm_group(0, wiou_sb, i_extra)
                o_ps = mm_group(2, wiou_sb, o_extra)
                u_ps = mm_group(4, wiou_sb, u_extra)
                i_sb = trans.tile([128, 2, P], f16, tag="isb", name="i_sb")
                o_sb = trans.tile([128, 2, P], f16, tag="osb", name="o_sb")
                act(i_sb, i_ps, Sig, biou_sb, (0, 1))
                act(o_sb, o_ps, Sig, biou_sb, (2, 3))

            u_sb = trans.tile([128, 2, P], f16, tag="usb", name="u_sb")
            act(u_sb, u_ps, Tanh, biou_sb, (4, 5))

            c_blk = c_l[:, :, bo : bo + P]
            nc.vector.tensor_mul(c_blk, i_sb, u_sb)

            if not leaf:
                if fuse4:
                    f_ps = pp.tile([128, 4, P], f32, tag="ps", name="f_ps")
                    for mj in range(2):
                        mm_fill(f_ps, mj, 0, wf_sb, f_extra(0))
                    for mj in range(2, 4):
                        mm_fill(f_ps, mj, 0, wf_sb, f_extra(1))
                    f4_sb = trans.tile([128, 4, P], f16, tag="fesb", name="f4_sb")
                    if has_bias:
                        for kk in range(4):
                            nc.scalar.activation(
                                out=f4_sb[:, kk, :], in_=f_ps[:, kk, :], func=Sig,
                                bias=bf_sb[:, kk % 2 : kk % 2 + 1])
                    else:
                        nc.scalar.activation(out=f4_sb, in_=f_ps, func=Sig)
                    fe_sb = f4_sb[:, 0:2, :]
                    fo_sb = f4_sb[:, 2:4, :]
                else:
                    fe_ps = mm_group(0, wf_sb, f_extra(0))
                    fo_ps = mm_group(0, wf_sb, f_extra(1))
                    fe_sb = trans.tile([128, 2, P], f16, tag="fesb", name="fe_sb")
                    fo_sb = trans.tile([128, 2, P], f16, tag="fosb", name="fo_sb")
                    act(fe_sb, fe_ps, Sig, bf_sb, (0, 1))
                    act(fo_sb, fo_ps, Sig, bf_sb, (0, 1))
                tm_e = trans.tile([128, 2, P], f16, tag="tme", name="tm_e")
                tm_o = trans.tile([128, 2, P], f16, tag="tmo", name="tm_o")
                for par, f_sb, tm in ((0, fe_sb, tm_e), (1, fo_sb, tm_o)):
                    for kk in range(2):
                        fv = f_sb[:, kk, :].rearrange("p (q b) -> p q b", b=bc)
                        tv = tm[:, kk, :].rearrange("p (q b) -> p q b", b=bc)
                        cv = child_view(c_child, kk, par)
                        nc.vector.tensor_mul(tv, fv, cv)
                nc.vector.tensor_add(c_blk, c_blk, tm_e)
                nc.vector.tensor_add(c_blk, c_blk, tm_o)

            t_sb = trans.tile([128, 2, P], f16, tag="tsb", name="t_sb")
            nc.scalar.activation(out=t_sb, in_=c_blk, func=Tanh)
            h_blk = h_l[:, :, bo : bo + P]
            nc.vector.tensor_mul(h_blk, o_sb, t_sb)

            if hs_out is not None:
                for kk in range(2):
                    hv = h_l[:, kk, bo : bo + P].rearrange(
                        "p (q two b) -> p q two b", two=2, b=bc
                    )
                    sv = hs_out[:, kk, bo // 2 : bo // 2 + P // 2].rearrange(
                        "p (q b) -> p q b", b=bc
                    )
                    nc.vector.tensor_add(sv, hv[:, :, 0, :], hv[:, :, 1, :])

        # Merged-phase tensors (levels < MERGE_LVL, batch nb).
        hm6 = big.tile([128, 2, 64 * nb], f16, tag="hm6", bufs=1)
        cm6 = big.tile([128, 2, 64 * nb], f16, tag="cm6", bufs=1)
        hs5 = big.tile([128, 2, 32 * nb], f16, tag="hs5", bufs=1)

        # ---- Phase 1: levels 9..6 per chunk, software-pipelined in diagonal
        # wave order: (ch, 9), then (ch+1, 9) with (ch, 8), etc.  Interleaving
        # chunk ch's small levels with chunk ch+1/ch+2's big levels keeps big
        # matmul groups in the PSUM ring while a small level's serial chain
        # drains, so PE/ACT never starve at chunk boundaries.
        state = {}

        def emit_p1_level(ch, lvl):
            h_prev, c_prev, hs_cur = state.get(ch, (None, None, None))
            n_l = 1 << lvl
            s_l = n_l - 1
            R = n_l * B_C
            xl = xpool.tile([128, 2, R], f16, tag=f"x{lvl}", name=f"x{lvl}")
            for kc in range(2):
                nc.sync.dma_start(
                    out=xl[:, kc, :],
                    in_=xt[ch, kc, :, s_l * B_C : (s_l + n_l) * B_C],
                )
            if lvl > MERGE_LVL:
                h_l = big.tile([128, 2, R], f16, tag=f"h{lvl}", name=f"h{lvl}")
                c_l = big.tile([128, 2, R], f16, tag=f"c{lvl}", name=f"c{lvl}")
            else:
                h_l = big.tile([128, 2, R], f16, tag="h6t", name="h6t")
                c_l = big.tile([128, 2, R], f16, tag="c6t", name="c6t")
            hs_next = None
            if lvl > MERGE_LVL:
                hs_next = big.tile(
                    [128, 2, R // 2], f16, tag=f"s{lvl - 1}", name=f"hs{lvl - 1}"
                )
            P = min(R, 512)
            for blk in range(R // P):
                emit_block(
                    P, blk * P, B_C,
                    (lambda xt_=xl, b_=blk, p_=P:
                     lambda kc: xt_[:, kc, b_ * p_ : (b_ + 1) * p_])(),
                    hs_cur, h_prev, c_prev, h_l, c_l, hs_next,
                )
            state[ch] = (h_l, c_l, hs_next)
            if lvl > MERGE_LVL:
                return
            # Level 6 done: scatter into the merged tensors and build the
            # merged level-5 child sums.  Merged column = q*nb + ch*B_C + b.
            for kk in range(2):
                hm_v = hm6.rearrange(
                    "p k (q e b) -> p k q e b", e=nch, b=B_C
                )[:, kk, :, ch, :]
                cm_v = cm6.rearrange(
                    "p k (q e b) -> p k q e b", e=nch, b=B_C
                )[:, kk, :, ch, :]
                h6v = h_l[:, kk, :].rearrange("p (q b) -> p q b", b=B_C)
                c6v = c_l[:, kk, :].rearrange("p (q b) -> p q b", b=B_C)
                nc.vector.tensor_copy(out=hm_v, in_=h6v)
                nc.vector.tensor_copy(out=cm_v, in_=c6v)
                hsv = hs5.rearrange(
                    "p k (q e b) -> p k q e b", e=nch, b=B_C
                )[:, kk, :, ch, :]
                h6p = h_l[:, kk, :].rearrange(
                    "p (q two b) -> p q two b", two=2, b=B_C
                )
                nc.vector.tensor_add(hsv, h6p[:, :, 0, :], h6p[:, :, 1, :])

        steps = [(ch, lvl) for ch in range(nch)
                 for lvl in range(DEPTH, MERGE_LVL - 1, -1)]
        steps.sort(key=lambda t: (t[0] + (DEPTH - t[1]), DEPTH - t[1]))
        for ch, lvl in steps:
            emit_p1_level(ch, lvl)

        # ---- Phase 2: merged levels 5..0 over all nb trees ----
        xm_sb = xpool.tile([128, 2, NM * nb], f16, tag="xm", bufs=1)
        for kc in range(2):
            nc.sync.dma_start(out=xm_sb[:, kc, :], in_=xm_d[kc])

        h_prev, c_prev, hs_cur = hm6, cm6, hs5
        for lvl in range(MERGE_LVL - 1, -1, -1):
            n_l = 1 << lvl
            s_l = n_l - 1
            R = n_l * nb
            h_l = big.tile([128, 2, R], f16, tag=f"mh{lvl % 2}", name=f"mh{lvl}")
            c_l = big.tile([128, 2, R], f16, tag=f"mc{lvl % 2}", name=f"mc{lvl}")
            hs_next = None
            if lvl > 0:
                hs_next = big.tile(
                    [128, 2, R // 2], f16, tag=f"ms{(lvl - 1) % 2}",
                    name=f"mhs{lvl - 1}",
                )
            P = min(R, 512)
            for blk in range(R // P):
                emit_block(
                    P, blk * P, nb,
                    (lambda lo=s_l * nb + blk * P, hi=s_l * nb + (blk + 1) * P:
                     lambda kc: xm_sb[:, kc, lo:hi])(),
                    hs_cur, h_prev, c_prev, h_l, c_l, hs_next,
                )
            h_prev, c_prev, hs_cur = h_l, c_l, hs_next

        h32 = trans.tile([128, 2, nb], f32, tag="h32", name="h32")
        c32 = trans.tile([128, 2, nb], f32, tag="c32", name="c32")
        nc.vector.tensor_copy(out=h32, in_=h_prev)
        nc.vector.tensor_copy(out=c32, in_=c_prev)
        for kc in range(2):
            nc.sync.dma_start(out=hout[kc][:, :], in_=h32[:, kc, :])
            nc.sync.dma_start(out=cout[kc][:, :], in_=c32[:, kc, :])

    nc.compile()
    return nc


def _get_nc(nch, has_bias):
    key = (nch, has_bias)
    if key not in _NC_CACHE:
        _NC_CACHE[key] = _build(nch, has_bias)
    return _NC_CACHE[key]


def _pack_inputs(x, W_iou, b_iou, U_iou, W_f, b_f, U_f, nch=NCH):
    """Host-side shard + layout prep. Returns (in_maps, has_bias)."""
    x = np.asarray(x, dtype=np.float32)
    nb = nch * B_C
    # [core, ch, b, node, d] -> [core, ch, d, node, b]
    xt = x.reshape(NCORES, NCH, B_C, NNODES, D)
    xt = np.ascontiguousarray(
        xt.transpose(0, 1, 4, 3, 2), dtype=np.float16
    ).reshape(NCORES, NCH, 2, 128, COLS)
    # merged upper-level x: [core, j, node<NM, d] -> [core, d, node, j]
    xm = x.reshape(NCORES, B_LOC, NNODES, D)[:, :nb, :NM, :]
    xm = np.ascontiguousarray(
        xm.transpose(0, 3, 2, 1), dtype=np.float16
    ).reshape(NCORES, 2, 128, NM * nb)

    wiou = np.ascontiguousarray(
        np.asarray(W_iou, np.float32).T, dtype=np.float16
    ).reshape(2, 128, 768)
    uiou = np.ascontiguousarray(
        np.asarray(U_iou, np.float32).T, dtype=np.float16
    ).reshape(2, 128, 768)
    wf = np.ascontiguousarray(
        np.asarray(W_f, np.float32).T, dtype=np.float16
    ).reshape(2, 128, 256)
    uf = np.ascontiguousarray(
        np.asarray(U_f, np.float32).T, dtype=np.float16
    ).reshape(2, 128, 256)

    b_iou = np.asarray(b_iou, np.float32)
    b_f = np.asarray(b_f, np.float32)
    has_bias = bool(np.any(b_iou) or np.any(b_f))

    in_maps = []
    for c in range(NCORES):
        m = {
            "xt": np.ascontiguousarray(xt[c, :nch]),
            "xm": xm[c],
            "wiou": wiou,
            "uiou": uiou,
            "wf": wf,
            "uf": uf,
        }
        if has_bias:
            m["biou"] = b_iou
            m["bf"] = b_f
        in_maps.append(m)
    return in_maps, has_bias


class _PjrtRunner:
    """Persistent-jit SPMD executor for a Bass program over 8 neuron devices.

    Mirrors concourse.bass2jax.run_bass_via_pjrt's multi-core branch, but
    keeps the compiled executable and device-resident inputs across calls so
    repeated executions (and timing runs) don't recompile or re-upload.
    """

    def __init__(self, nc):
        import jax
        import concourse.mybir as mybir
        from concourse.bass2jax import _bass_exec_p, install_neuronx_cc_hook
        from jax.sharding import Mesh, NamedSharding, PartitionSpec
        from jax.experimental.shard_map import shard_map

        install_neuronx_cc_hook()
        assert nc.partition_id_tensor is None

        self.jax = jax
        in_names, out_names, out_avals = [], [], []
        for alloc in nc.m.functions[0].allocations:
            if not isinstance(alloc, mybir.MemoryLocationSet):
                continue
            name = alloc.memorylocations[0].name
            if alloc.kind == "ExternalInput":
                in_names.append(name)
            elif alloc.kind == "ExternalOutput":
                out_names.append(name)
                out_avals.append(
                    jax.core.ShapedArray(
                        tuple(alloc.tensor_shape), mybir.dt.np(alloc.dtype)
                    )
                )
        self.in_names, self.out_names, self.out_avals = in_names, out_names, out_avals
        n_params = len(in_names)
        n_outs = len(out_names)
        all_in = in_names + out_names

        def _body(*args):
            return tuple(
                _bass_exec_p.bind(
                    *args,
                    out_avals=tuple(out_avals),
                    in_names=tuple(all_in),
                    out_names=tuple(out_names),
                    lowering_input_output_aliases=(),
                    sim_require_finite=True,
                    sim_require_nnan=True,
                    nc=nc,
                )
            )

        devices = jax.devices()[:NCORES]
        self.mesh = Mesh(np.asarray(devices), ("core",))
        spec = PartitionSpec("core")
        self.sharding = NamedSharding(self.mesh, spec)
        donate = tuple(range(n_params, n_params + n_outs))
        self.fn = jax.jit(
            shard_map(
                _body,
                mesh=self.mesh,
                in_specs=(spec,) * (n_params + n_outs),
                out_specs=(spec,) * n_outs,
                check_rep=False,
            ),
            donate_argnums=donate,
            keep_unused=True,
        )
        self.dev_inputs = None

    def put_inputs(self, in_maps):
        jax = self.jax
        concat = [
            np.concatenate([np.asarray(m[nm]) for m in in_maps], axis=0)
            for nm in self.in_names
        ]
        self.dev_inputs = [jax.device_put(a, self.sharding) for a in concat]
        for a in self.dev_inputs:
            a.block_until_ready()

    def _zero_outs(self):
        jax = self.jax
        zs = [
            jax.device_put(
                np.zeros((NCORES * av.shape[0], *av.shape[1:]), av.dtype),
                self.sharding,
            )
            for av in self.out_avals
        ]
        for z in zs:
            z.block_until_ready()
        return zs

    def run(self):
        outs = self.fn(*self.dev_inputs, *self._zero_outs())
        return {
            nm: np.asarray(outs[i]).reshape(NCORES, *self.out_avals[i].shape)
            for i, nm in enumerate(self.out_names)
        }

    def time_runs(self, n=5):
        import time

        times = []
        for _ in range(n):
            zs = self._zero_outs()
            t0 = time.perf_counter()
            outs = self.fn(*self.dev_inputs, *zs)
            for o in outs:
                o.block_until_ready()
            times.append(time.perf_counter() - t0)
        return times


_RUNNERS = {}


def _get_runner(nch, has_bias):
    key = (nch, has_bias)
    if key not in _RUNNERS:
        _RUNNERS[key] = _PjrtRunner(_get_nc(nch, has_bias))
    return _RUNNERS[key]


def kernel(x, W_iou, b_iou, U_iou, W_f, b_f, U_f):
    in_maps, has_bias = _pack_inputs(x, W_iou, b_iou, U_iou, W_f, b_f, U_f)
    runner = _get_runner(NCH, has_bias)
    runner.put_inputs(in_maps)
    res = runner.run()
    LAST["runner"] = runner

    h = np.empty((B, D), np.float32)
    c = np.empty((B, D), np.float32)
    for i in range(NCORES):
        h[i * B_LOC : (i + 1) * B_LOC] = res["hout"][i].reshape(D, B_LOC).T
        c[i * B_LOC : (i + 1) * B_LOC] = res["cout"][i].reshape(D, B_LOC).T
    return h, c

